# revision 1
# baseline (speedup 1.0000x reference)
"""ChebyKAN layer on 8 Trainium2 NeuronCores.

y[b,o] = sum_{i,d} T_d(tanh(x[b,i])) * coeffs[i,o,d]

The Chebyshev basis is re-parameterized (exact 9x9 linear transform of the
coefficients on host) into products of Chebyshev values that the device can
build with 4 ACT squares + 3 DVE fused ops per element:
  G0=1, G1=t, G2=t^2, G3q=(G2-3/4)t=T3/4, G4=(2G2-1)^2=T2^2,
  G5q=(G4-1/2)t=(T5+T3)/4, G6=(4G3q)^2=T3^2, G7q=(G6-1/2)t=(T7+T5)/4,
  G8=(2G4-1)^2=T4^2
The einsum is a (B x 9216) @ (9216 x 1024) matmul in fp32r (full PE rate,
~FP22 precision), accumulated in PSUM over all 72 contraction blocks.

Sharding: data-parallel over batch (2048 rows/core), coeffs replicated.
"""

import numpy as np
import concourse.mybir as mybir
import concourse.tile as tile
from concourse import bacc
from concourse.bass_utils import run_bass_kernel_spmd

B, I, O, D1 = 16384, 1024, 1024, 9
CORES = 8
BC = B // CORES            # 2048 batch rows per core
P = 128
MACRO = 1024               # batch rows per generation
OH = 512                   # output cols per generation
N_M = BC // MACRO          # 2
N_OH = O // OH             # 2
IB = I // P                # 8 i-blocks
BT = MACRO // P            # 8 batch subtiles per macro

F32 = mybir.dt.float32
F32R = mybir.dt.float32r
AF = mybir.ActivationFunctionType
OP = mybir.AluOpType

_CACHE = {}
_last_in_maps = None

# G_k = sum_d M[k,d] T_d  (exact); host solves M^T C' = C
_M = np.zeros((9, 9))
_M[0, 0] = 1; _M[1, 1] = 1
_M[2, 0] = .5; _M[2, 2] = .5
_M[3, 3] = .25
_M[4, 0] = .5; _M[4, 4] = .5
_M[5, 3] = .25; _M[5, 5] = .25
_M[6, 0] = .5; _M[6, 6] = .5
_M[7, 5] = .25; _M[7, 7] = .25
_M[8, 0] = .5; _M[8, 8] = .5
_A = np.linalg.inv(_M.T)


def _emit(nc, xp, tp, g4p, wp, c2p, op_, pp, xt_d, c2_d, y_d, ones, neg1, rep):
    t_tiles = {}
    for m in range(N_M):
        for oh in range(N_OH):
            psum = [
                pp.tile([P, OH], F32, tag=f"ps{bt}", name=f"ps{bt}_{rep}_{m}_{oh}")
                for bt in range(BT)
            ]
            g = {}
            sfx = f"{rep}_{m}_{oh}"

            def mm_level(d, tiles, start=False, stop=False):
                for ib in range(IB):
                    c2t = c2p.tile(
                        [P, OH], F32R, tag="c2", name=f"c2_{sfx}_{d}_{ib}"
                    )
                    r0 = (d * IB + ib) * P
                    nc.sync.dma_start(
                        c2t[:],
                        c2_d[r0:r0 + P, oh * OH:(oh + 1) * OH].bitcast(F32R),
                    )
                    for bt in range(BT):
                        lhs = (
                            ones[:]
                            if tiles is None
                            else tiles[ib][:, bt * P:(bt + 1) * P]
                        )
                        nc.tensor.matmul(
                            psum[bt][:],
                            lhs,
                            c2t[:],
                            start=(start and ib == 0),
                            stop=(stop and ib == IB - 1),
                        )

            def wtile(lvl, ib):
                w = wp.tile(
                    [P, MACRO], F32R, tag=f"w{ib}", name=f"w{lvl}_{ib}_{sfx}"
                )
                g.setdefault(lvl, {})[ib] = w
                return w

            # level 0: ones
            mm_level(0, None, start=True)
            # level 1: t = tanh(x), persists across oh
            if oh == 0:
                for ib in range(IB):
                    xt = xp.tile([P, MACRO], F32, tag="xt", name=f"xt{rep}_{m}_{ib}")
                    nc.sync.dma_start(
                        xt[:], xt_d[ib * P:(ib + 1) * P, m * MACRO:(m + 1) * MACRO]
                    )
                    t = tp.tile([P, MACRO], F32R, tag=f"t{ib}", name=f"t{ib}_{rep}_{m}")
                    nc.scalar.activation(t[:], xt[:], AF.Tanh)
                    t_tiles[ib] = t
            mm_level(1, t_tiles)
            # level 2: G2 = t^2
            for ib in range(IB):
                nc.scalar.activation(wtile(2, ib)[:], t_tiles[ib][:], AF.Square)
            mm_level(2, g[2])
            # level 4: G4 = (2*G2 - 1)^2   [pinned tag]
            for ib in range(IB):
                w = g4p.tile([P, MACRO], F32R, tag=f"g4{ib}", name=f"g4_{ib}_{sfx}")
                nc.scalar.activation(w[:], g[2][ib][:], AF.Square, bias=neg1[:], scale=2.0)
                g.setdefault(4, {})[ib] = w
            mm_level(4, g[4])
            # level 3: G3q = (G2 - 0.75) * t
            for ib in range(IB):
                nc.vector.scalar_tensor_tensor(
                    wtile(3, ib)[:], g[2][ib][:], 0.75, t_tiles[ib][:],
                    OP.subtract, OP.mult,
                )
            mm_level(3, g[3])
            # level 6: G6 = (4*G3q)^2
            for ib in range(IB):
                nc.scalar.activation(wtile(6, ib)[:], g[3][ib][:], AF.Square, scale=4.0)
            mm_level(6, g[6])
            # level 5: G5q = (G4 - 0.5) * t
            for ib in range(IB):
                nc.vector.scalar_tensor_tensor(
                    wtile(5, ib)[:], g[4][ib][:], 0.5, t_tiles[ib][:],
                    OP.subtract, OP.mult,
                )
            mm_level(5, g[5])
            # level 7: G7q = (G6 - 0.5) * t
            for ib in range(IB):
                nc.vector.scalar_tensor_tensor(
                    wtile(7, ib)[:], g[6][ib][:], 0.5, t_tiles[ib][:],
                    OP.subtract, OP.mult,
                )
            mm_level(7, g[7])
            # level 8: G8 = (2*G4 - 1)^2
            for ib in range(IB):
                nc.scalar.activation(
                    wtile(8, ib)[:], g[4][ib][:], AF.Square, bias=neg1[:], scale=2.0
                )
            mm_level(8, g[8], stop=True)

            for bt in range(BT):
                ob = op_.tile([P, OH], F32, tag="ob", name=f"ob_{sfx}_{bt}")
                nc.vector.tensor_copy(ob[:], psum[bt][:])
                nc.sync.dma_start(
                    y_d[
                        m * MACRO + bt * P:m * MACRO + (bt + 1) * P,
                        oh * OH:(oh + 1) * OH,
                    ],
                    ob[:],
                )


def build_nc(reps=1):
    nc = bacc.Bacc("TRN2", target_bir_lowering=False, debug=False, num_devices=CORES)
    xt_d = nc.dram_tensor("xt", [I, BC], F32, kind="ExternalInput")
    c2_d = nc.dram_tensor("c2", [D1 * I, O], F32, kind="ExternalInput")
    y_d = nc.dram_tensor("y", [BC, O], F32, kind="ExternalOutput")

    with tile.TileContext(nc) as tc:
        with (
            tc.tile_pool(name="xp", bufs=1) as xp,       # x staging
            tc.tile_pool(name="cp", bufs=1) as cp,       # constants
            tc.tile_pool(name="tp", bufs=1) as tp,       # tanh tiles (persist per m)
            tc.tile_pool(name="g4p", bufs=1) as g4p,     # pinned G4 level
            tc.tile_pool(name="wp", bufs=3) as wp,       # G sliding window
            tc.tile_pool(name="c2p", bufs=3) as c2p,     # coeff stream
            tc.tile_pool(name="op", bufs=2) as op_,      # psum eviction staging
            tc.tile_pool(name="pp", bufs=1, space="PSUM") as pp,
        ):
            xboot = xp.tile([P, MACRO], F32, tag="xt")
            nc.sync.dma_start(xboot[:, 0:P], xt_d[0:P, 0:P])
            ones = cp.tile([P, P], F32R, tag="ones")
            nc.vector.tensor_scalar(ones[:], xboot[:, 0:P], 0.0, 1.0, OP.mult, OP.add)
            neg1 = cp.tile([P, 1], F32, tag="neg1")
            nc.vector.memset(neg1[:], -1.0)

            for rep in range(reps):
                _emit(nc, xp, tp, g4p, wp, c2p, op_, pp, xt_d, c2_d, y_d, ones, neg1, rep)
    nc.compile()
    return nc


def kernel(x: np.ndarray, cheby_coeffs: np.ndarray) -> np.ndarray:
    assert x.shape == (B, I) and cheby_coeffs.shape == (I, O, D1)
    if "nc" not in _CACHE:
        _CACHE["nc"] = build_nc()
    nc = _CACHE["nc"]

    xt = np.ascontiguousarray(x.T.astype(np.float32, copy=False))          # (I, B)
    cp = np.einsum("ed,iod->ioe", _A, cheby_coeffs.astype(np.float64))     # C' transform
    c2 = np.ascontiguousarray(
        np.transpose(cp, (2, 0, 1)).reshape(D1 * I, O).astype(np.float32)
    )
    in_maps = [
        {"xt": np.ascontiguousarray(xt[:, c * BC:(c + 1) * BC]), "c2": c2}
        for c in range(CORES)
    ]
    global _last_in_maps
    _last_in_maps = in_maps
    res = run_bass_kernel_spmd(nc, in_maps, core_ids=list(range(CORES)))
    return np.concatenate([res.results[c]["y"] for c in range(CORES)], axis=0)

